# revision 48
# baseline (speedup 1.0000x reference)
"""Trainium2 Bass kernel for nn_MultiHeadSelfAttention (B=2, N=2048, C=1024, H=16).

Sharding: 8 cores = (batch b in {0,1}) x (head-group g in {0..3}); each core
computes 4 heads of one batch plus its partial output projection. The host
sums the 4 partial projections per batch and adds the bias constant
(v-bias and proj-bias folded together; k-bias is softmax-invariant and
dropped; q-bias applied on device).

Layouts (all transposed so no on-chip transposes needed):
  qT,kT [256,2048] = W_{q,k} @ x^T   (bf16)
  v     [2048,256] = x @ W_v^T       (fp8e4 + fp8 residual rows)
  S'_h  [kv,q] = K_h q_h^T           (K=64 row-tiled pairs, concurrent on PE)
  P'    = exp(S')                    (ACT, psum->sbuf, fp8e4)
  O^T_h [VD,q] = V_h^T P'_h          (fp8 DoubleRow: 2 kv-chunks per matmul,
                                      rows 0:64 = O, 64:96 = V-residual
                                      correction, 96 = softmax denominator)

Schedule: the attention sweeps s=(n,hp) run as 8 pipelined slots per rep;
slot t interleaves, per kv chunk i, the QK+exp of sweep t with the AV +
denominator matmuls of sweep t-1 (slot 0 consumes the previous rep's last
sweep), so the Scalar engine (exp) never drains. The QKV projection of rep
r+1 and the y-projection units ride the slots as fine-grained extra matmuls,
with q/k/v and O buffers double-buffered by rep parity. The rep loop is a
hardware For_i over an unrolled parity pair; a prologue rep (internal lag)
fills the pipeline and an epilogue drains the last sweep.
"""
import sys
import os

sys.path.insert(0, "/opt/trn_rl_repo")

import numpy as np
import ml_dtypes

import concourse.bass as bass
import concourse.mybir as mybir
from concourse import bacc
from concourse.tile import TileContext
from concourse import bass_utils
from concourse.bass_utils import run_bass_kernel_spmd

if os.environ.get("KLDWOPT", "0") == "1":
    # walrus ldw-opt: let the backend dedupe/hoist LDWEIGHTS
    _orig_run_command = bass_utils.run_command

    def _patched_run_command(argv, **kwargs):
        argv = ["--enable-ldw-opt=true" if a == "--enable-ldw-opt=false" else a
                for a in argv]
        return _orig_run_command(argv, **kwargs)

    bass_utils.run_command = _patched_run_command

F32R = mybir.dt.float32r
F32 = mybir.dt.float32
BF16 = mybir.dt.bfloat16
MM_BF16 = os.environ.get("KMMDT", "bf16") == "bf16"
MMDT = BF16 if MM_BF16 else F32R
# DoubleRow AV: P (exp scores) and V in fp8e4, kv-chunks paired so each AV
# matmul contracts 256 kv (2 fp8 weights/cell, 2 mul/cycle) — halves AV's
# PE stream time. Requires fp8 P+V (adds ~1.5e-2 rel err vs bf16's 2.5e-3;
# gate is 2e-2).
DRAV = os.environ.get("KDRAV", "1") == "1"
P8 = DRAV or os.environ.get("KP8", "0") == "1"
PDT = mybir.dt.float8e4 if P8 else BF16
DR = mybir.MatmulPerfMode.DoubleRow
# V-residual: AV lhsT rows 64:96 carry res = v - fp8(v) for features 0:31,
# added back to O at normalize time — free in the matmul (cost ~ N columns,
# independent of M); halves the fp8-V error variance. Row 96 = ones (D).
# 32-row blocks keep every DVE read quadrant-aligned (walrus requirement).
# VRES on: with AV front-loading, the VD=100 + residual config measured
# fastest (182.0/184.6us vs 189.1 at VD=80) — the lighter finish_sweep at
# VD=80 interacts badly with the early finish; and VRES buys rel err
# 1.56e-2 vs 1.72e-2 (gate 2e-2).
VRES = DRAV and os.environ.get("KVRES", "1") == "1"
# VD: lhsT free size per pair half (64 v-feats + ones + pad to a 16B-aligned
# pair stride: 4*VD % 16 == 0). Smaller VD = fewer LDWEIGHTS columns.
VD = 100 if VRES else int(os.environ.get("KVD", "80"))
ONES_COL = 96 if VRES else 64
# GPSIMD cannot access PSUM (walrus BIR rule) — evictions read PSUM, so
# they must stay on DVE.
GPEV = os.environ.get("KGPEV", "0") == "1"
# Split per-head [128,512] exps + psA=5x1-bank + psC=1: measured 297us on HW
# (vs 184.6) — the 1-deep psC ring stalls the interleaved projection chains.
# Keep off.
EXPS = DRAV and os.environ.get("KEXPS", "0") == "1"
PYL = os.environ.get("KPYL", "1") == "1"
# SBUF->SBUF partition-broadcast DMA is illegal (zero partition step) — the
# DRAM round trip through dscr is required for the denominator broadcast.
SBBC = os.environ.get("KSBBC", "0") == "1"
Exp = mybir.ActivationFunctionType.Exp

B, N, C, H = 2, 2048, 1024, 16
HD = C // H          # 64
SCALE = 1.0 / np.sqrt(HD).astype(np.float32)

NQ = N // 512        # 4 q-chunks of 512
NK = N // 128        # 16 kv-chunks of 128
NJ = C // 128        # 8 contraction chunks for projections


def build_nc():
    part = os.environ.get("KPART", "full")
    reps = int(os.environ.get("KREPS", "1"))
    nc = bacc.Bacc("TRN2", target_bir_lowering=False, debug=False, num_devices=8)

    xt_d = nc.dram_tensor("xt", [C, N], MMDT, kind="ExternalInput").ap()
    wqk_d = nc.dram_tensor("wqk", [128, NJ, 512], MMDT, kind="ExternalInput").ap()
    wv_d = nc.dram_tensor("wv", [128, NJ, 256], MMDT, kind="ExternalInput").ap()
    qb_d = nc.dram_tensor("qb", [128, 2], F32, kind="ExternalInput").ap()
    pw_d = nc.dram_tensor("pw", [128, 2, 1024], MMDT, kind="ExternalInput").ap()
    one_d = nc.dram_tensor("onec", [128, 1], PDT, kind="ExternalInput").ap()
    if DRAV:
        # per (chunk-pair, pair, head): feature tail 64:VD = [1, 0*(VD-65)]
        vones_d = nc.dram_tensor("vones", [128, NK * 4, VD - 64], PDT,
                                 kind="ExternalInput").ap()
    else:
        vones_d = nc.dram_tensor("vones", [128, NK, 4, 1], PDT,
                                 kind="ExternalInput").ap()
    out_d = nc.dram_tensor("out", [N, C], F32, kind="ExternalOutput").ap()

    ilv = reps > 1  # interleave next-rep projection into the slots

    with TileContext(nc) as tc:
        with tc.tile_pool(name="const", bufs=1) as const, \
             tc.tile_pool(name="persist", bufs=1) as persist, \
             tc.tile_pool(name="xs", bufs=16) as xs, \
             tc.tile_pool(name="pts", bufs=28) as pts, \
             tc.tile_pool(name="dsbp", bufs=1) as dsbp, \
             tc.tile_pool(name="dbp", bufs=1) as dbp, \
             tc.tile_pool(name="rbp", bufs=1) as rbp, \
             tc.tile_pool(name="oup", bufs=2) as oup, \
             tc.tile_pool(name="yts", bufs=3) as yts, \
             tc.tile_pool(name="scr", bufs=1, space="DRAM") as scr, \
             tc.tile_pool(name="psA", bufs=(5 if EXPS else 2),
                          space="PSUM") as psA, \
             tc.tile_pool(name="psB", bufs=2, space="PSUM") as psB, \
             tc.tile_pool(name="psC", bufs=(1 if EXPS else 2),
                          space="PSUM") as psC:

            wqk_t = const.tile([128, NJ, 512], MMDT)
            wv_t = const.tile([128, NJ, 256], MMDT)
            qb_t = const.tile([128, 2], F32)
            pw_t = const.tile([128, 2, 1024], MMDT)
            ones_t = const.tile([128, 1], PDT)

            npar = 2 if ilv else 1
            q_p = [persist.tile([128, 2, N], MMDT, name=f"q_all{i_}")
                   for i_ in range(npar)]
            k_p = [persist.tile([128, 2, N], MMDT, name=f"k_all{i_}")
                   for i_ in range(npar)]
            # [kv, chunk, head(4), 64 v-feats + ones]: the ones column makes
            # the AV matmul (M=65) emit the softmax denominator as row 64.
            # DoubleRow: [kv, chunk-pair, pair, head, VD] — 64 feats + ones +
            # (residual rows | pad); pair stride (4*VD B) stays 16B-aligned.
            if DRAV:
                v_p = [persist.tile([128, NK // 2, 2, 4, VD], PDT,
                                    name=f"v_all{i_}") for i_ in range(npar)]
            else:
                v_p = [persist.tile([128, NK, 4, 65], PDT, name=f"v_all{i_}")
                       for i_ in range(npar)]
            on_p = [persist.tile([128, 2, N], MMDT, name=f"on_all{i_}")
                    for i_ in range(npar)]
            # slot-7 exp output crosses the For_i back edge (consumed by the
            # next rep's slot 0), so it lives in a persistent ring, not a pool
            if DRAV:
                pt7 = persist.tile([128, NK // 2, 2, 1024], PDT, name="pt7")
            else:
                pt7 = persist.tile([128, NK, 1024], PDT, name="pt7")
            dscr = [scr.tile([2, NQ, 2, 512], F32, name=f"dscr{i_}")
                    for i_ in range(npar)]

            # constants load once
            nc.scalar.dma_start(out=qb_t, in_=qb_d)
            for par_ in range(npar):
                if DRAV:
                    nc.scalar.dma_start(
                        out=v_p[par_][:, :, :, :, 64:VD].rearrange(
                            "p a b c d -> p (a b c) d"),
                        in_=vones_d)
                else:
                    nc.scalar.dma_start(out=v_p[par_][:, :, :, 64:65],
                                        in_=vones_d)
            nc.scalar.dma_start(out=ones_t, in_=one_d)
            nc.scalar.dma_start(out=pw_t, in_=pw_d)
            for j in range(NJ):
                nc.scalar.dma_start(out=wqk_t[:, j, :], in_=wqk_d[:, j, :])
                nc.scalar.dma_start(out=wv_t[:, j, :], in_=wv_d[:, j, :])

            def proj_loads(n):
                xts = []
                for j in range(NJ):
                    xt_t = xs.tile([128, 512], MMDT, tag="xt")
                    nc.sync.dma_start(
                        out=xt_t,
                        in_=xt_d[128 * j:128 * (j + 1), 512 * n:512 * (n + 1)])
                    xts.append(xt_t)
                return xts

            def proj_round_thunks(n, r, xts, par):
                """Chain r (of 8) of chunk n: ONE accumulation chain in ONE
                PSUM bank, as single-MM thunks + an eviction thunk. Single-
                bank chains alternate psC bufs, so each chain's eviction
                hides behind the next chain instead of head-blocking it."""
                nsl = slice(512 * n, 512 * (n + 1))
                c = psC.tile([128, 512], F32, tag="psC", name=f"pr{r}")
                thunks = []
                for j in range(NJ):
                    st, sp = (j == 0), (j == NJ - 1)
                    if r < 4:
                        base = 128 * r

                        def mm(j=j, base=base, st=st, sp=sp):
                            nc.tensor.matmul(
                                c, lhsT=wqk_t[:, j, base:base + 128],
                                rhs=xts[j], start=st, stop=sp)
                    else:
                        t_ = r - 4

                        def mm(j=j, t_=t_, st=st, sp=sp):
                            nc.tensor.matmul(
                                c[:, 0:256],
                                lhsT=xts[j][:, 128 * t_:128 * (t_ + 1)],
                                rhs=wv_t[:, j, :], start=st, stop=sp)
                    thunks.append(mm)

                # proj evictions run on GpSimd (otherwise idle): keeps DVE
                # free for the normalize chain and unblocks the psC ring
                # sooner when DVE's queue is backed up.
                ev = nc.gpsimd if GPEV else nc.vector

                def evict():
                    if r < 2:
                        ev.tensor_scalar_add(
                            out=q_p[par][:, r, nsl], in0=c,
                            scalar1=qb_t[:, r:r + 1])
                    elif r < 4:
                        ev.tensor_copy(out=k_p[par][:, r - 2, nsl],
                                       in_=c)
                    else:
                        j_ = 4 * n + (r - 4)
                        dst = (v_p[par][:, j_ // 2, j_ % 2, :, 0:64]
                               if DRAV else v_p[par][:, j_, :, 0:64])
                        src = c[:, 0:256].rearrange("p (a b) -> p a b", a=4)
                        ev.tensor_copy(out=dst, in_=src)
                        if VRES:
                            vsl = v_p[par][:, j_ // 2, j_ % 2]
                            ev.tensor_sub(
                                out=vsl[:, :, 64:96],
                                in0=src[:, :, 0:32],
                                in1=vsl[:, :, 0:32])
                thunks.append(evict)
                return thunks

            def py_unit_thunk(pyn, u, par):
                def run():
                    m = 4 * pyn + u // 2
                    nn = u % 2
                    py = psC.tile([128, 512], F32, tag="psC", name="py")
                    for hp in range(2):
                        nc.tensor.matmul(
                            py, lhsT=on_p[par][:, hp, 128 * m:128 * (m + 1)],
                            rhs=pw_t[:, hp, 512 * nn:512 * (nn + 1)],
                            start=(hp == 0), stop=(hp == 1))
                    yt = yts.tile([128, 512], F32, tag="yt")
                    nc.vector.tensor_copy(out=yt, in_=py)
                    nc.sync.dma_start(
                        out=out_d[128 * m:128 * (m + 1),
                                  512 * nn:512 * (nn + 1)],
                        in_=yt)
                return run

            def finish_sweep(pn, php, par, oe_ps, oo_ps):
                pnsl = slice(512 * pn, 512 * (pn + 1))
                # bank A's readers (ou rows + D row) run first on DVE so the
                # next slot's first AV matmul (head-even -> bank A) unblocks
                # ~0.7us earlier; bank B's reader follows
                ou = oup.tile([128, 512], F32, tag="ou")
                dsb = dsbp.tile([128, 512], F32, tag="dsb")
                osh = dsbp.tile([128, 512], F32, tag="osh")
                if VRES:
                    # O = hi + res (features 0:31); res rows live at PSUM
                    # partitions 64:96, staged through an aligned copy (a
                    # partition-shifted two-input add is illegal).
                    rse = dsbp.tile([128, 512], F32, tag="rse")
                    nc.vector.tensor_copy(out=rse[0:32, :], in_=oe_ps[64:96, :])
                    nc.vector.tensor_add(out=ou[0:32, :], in0=oe_ps[0:32, :],
                                         in1=rse[0:32, :])
                    nc.vector.tensor_copy(out=ou[32:64, :], in_=oe_ps[32:64, :])
                    nc.vector.tensor_copy(out=dsb[64:65, :], in_=oe_ps[96:97, :])
                    rso = dsbp.tile([128, 512], F32, tag="rso")
                    nc.vector.tensor_copy(out=rso[0:32, :], in_=oo_ps[64:96, :])
                    nc.vector.tensor_add(out=osh[0:32, :], in0=oo_ps[0:32, :],
                                         in1=rso[0:32, :])
                    nc.vector.tensor_copy(out=osh[32:64, :], in_=oo_ps[32:64, :])
                    nc.vector.tensor_copy(out=osh[64:65, :], in_=oo_ps[96:97, :])
                else:
                    nc.vector.tensor_copy(out=ou[0:64, :], in_=oe_ps[0:64, :])
                    nc.vector.tensor_copy(out=dsb[64:65, :], in_=oe_ps[64:65, :])
                    nc.vector.tensor_copy(out=osh[0:65, :], in_=oo_ps[0:65, :])
                nc.sync.dma_start(out=ou[64:128, :], in_=osh[0:64, :])
                db = dbp.tile([128, 512], F32, tag="db")
                if SBBC:
                    # broadcast the denominator rows SBUF->SBUF directly
                    nc.sync.dma_start(
                        out=db[0:64, :],
                        in_=dsb[64:65, :].to_broadcast([64, 512]))
                    nc.sync.dma_start(
                        out=db[64:128, :],
                        in_=osh[64:65, :].to_broadcast([64, 512]))
                else:
                    nc.sync.dma_start(out=dscr[par][php, pn, 0, :],
                                      in_=dsb[64:65, :])
                    nc.sync.dma_start(out=dscr[par][php, pn, 1, :],
                                      in_=osh[64:65, :])
                    nc.sync.dma_start(
                        out=db[0:64, :],
                        in_=dscr[par][php, pn, 0:1, :].to_broadcast([64, 512]))
                    nc.sync.dma_start(
                        out=db[64:128, :],
                        in_=dscr[par][php, pn, 1:2, :].to_broadcast([64, 512]))
                rb = rbp.tile([128, 512], F32, tag="rb")
                nc.vector.reciprocal_approx_fast(out=rb, in_=db)
                # keep the multiply on DVE: GpSimd's per-op dispatch overhead
                # made it a net loss (measured +~10us)
                nc.vector.tensor_mul(out=on_p[par][:, php, pnsl], in0=ou,
                                     in1=rb)

            def emit_av_group(prev, ppar, ptp, i, oe_ps, oo_ps):
                pn, php = prev
                if DRAV:
                    # ptp: [128, 2, 1024] fp8 pair tile; i is the pair index.
                    st, sp = (i == 0), (i == NK // 2 - 1)
                    nc.tensor.matmul(oe_ps[0:VD, :],
                                     lhsT=v_p[ppar][:, i, :, 2 * php, 0:VD],
                                     rhs=ptp[:, :, 0:512],
                                     start=st, stop=sp, perf_mode=DR)
                    nc.tensor.matmul(oo_ps[0:VD, :],
                                     lhsT=v_p[ppar][:, i, :, 2 * php + 1, 0:VD],
                                     rhs=ptp[:, :, 512:1024],
                                     start=st, stop=sp, perf_mode=DR)
                    return
                st, sp = (i == 0), (i == NK - 1)
                nc.tensor.matmul(oe_ps[0:65, :],
                                 lhsT=v_p[ppar][:, i, 2 * php, 0:65],
                                 rhs=ptp[:, 0:512], start=st, stop=sp)
                nc.tensor.matmul(oo_ps[0:65, :],
                                 lhsT=v_p[ppar][:, i, 2 * php + 1, 0:65],
                                 rhs=ptp[:, 512:1024], start=st, stop=sp)

            def emit_slots(p, pt_wrap, wrap_prev):
                """8 pipelined slots of one rep (parity p).

                pt_wrap/wrap_prev: pt list + (sweep, parity) whose AV runs in
                slot 0 (the previous rep's sweep 7), or None for internal lag
                (prologue: slot 0 has no AV).
                Returns the pt list + sweep of slot 7 (for the next rep/drain).
                """
                pt_prev, prev = pt_wrap, wrap_prev
                for t in range(8):
                    cur = (t // 2, t % 2)
                    n, hp = cur
                    nsl = slice(512 * n, 512 * (n + 1))
                    # extras for this slot
                    extras = []
                    if ilv:
                        ch = t // 2   # proj chunk rides slots 2ch, 2ch+1
                        if t % 2 == 0:
                            if getattr(emit_slots, "pre_ch", None) == ch:
                                xts_c = emit_slots.pre
                            else:
                                xts_c = proj_loads(ch)
                            emit_slots.xts = xts_c
                            for r in range(6):
                                extras += proj_round_thunks(ch, r, xts_c, 1 - p)
                        else:
                            for r in (6, 7):
                                extras += proj_round_thunks(ch, r,
                                                            emit_slots.xts,
                                                            1 - p)
                            if ch < 3:
                                # prefetch next chunk's x tiles a slot early
                                # so its first proj matmul never waits on DMA
                                emit_slots.pre = proj_loads(ch + 1)
                                emit_slots.pre_ch = ch + 1
                    if PYL:
                        # y-proj groups shifted 2 slots later than their
                        # earliest-ready point: pyn 0,1 at slots 5,7 (>=3
                        # slots after their on_p normalize completes), pyn
                        # 2,3 ride the NEXT rep's slots 1,3 (a full rep of
                        # slack) — the old placement was exactly just-in-
                        # time behind the dscr DRAM round trip.
                        if t in (1, 3) and wrap_prev is not None:
                            for u in range(8):
                                extras.append(py_unit_thunk(
                                    2 + (t - 1) // 2, u, wrap_prev[2]))
                        if t in (5, 7):
                            for u in range(8):
                                extras.append(py_unit_thunk((t - 5) // 2,
                                                            u, p))
                    else:
                        if t == 1 and wrap_prev is not None:
                            for u in range(8):
                                extras.append(py_unit_thunk(3, u,
                                                            wrap_prev[2]))
                        if t in (3, 5, 7):
                            for u in range(8):
                                extras.append(py_unit_thunk((t - 3) // 2,
                                                            u, p))
                    ne = len(extras)

                    pt_cur = []
                    oe_ps = oo_ps = None
                    ptp = None
                    fin_done = False
                    for i in range(16):
                        isl = slice(128 * i, 128 * (i + 1))
                        if EXPS:
                            s2e = psA.tile([128, 512], F32, tag="psA",
                                           name="s2e")
                            s2o = psA.tile([128, 512], F32, tag="psA",
                                           name="s2o")
                            nc.tensor.matmul(s2e,
                                             lhsT=k_p[p][0:64, hp, isl],
                                             rhs=q_p[p][0:64, hp, nsl],
                                             start=True, stop=True)
                            nc.tensor.matmul(s2o,
                                             lhsT=k_p[p][64:128, hp, isl],
                                             rhs=q_p[p][64:128, hp, nsl],
                                             start=True, stop=True)
                        else:
                            s2 = psA.tile([128, 1024], F32, tag="psA",
                                          name="s2")
                            nc.tensor.matmul(s2[:, 0:512],
                                             lhsT=k_p[p][0:64, hp, isl],
                                             rhs=q_p[p][0:64, hp, nsl],
                                             start=True, stop=True)
                            nc.tensor.matmul(s2[:, 512:1024],
                                             lhsT=k_p[p][64:128, hp, isl],
                                             rhs=q_p[p][64:128, hp, nsl],
                                             start=True, stop=True)
                        if DRAV:
                            if i % 2 == 0:
                                if t == 7:
                                    ptp = pt7[:, i // 2]
                                else:
                                    ptp = pts.tile([128, 2, 1024], PDT,
                                                   tag="pt")
                                pt_cur.append(ptp)
                            pt = ptp[:, i % 2, :]
                        elif t == 7:
                            pt = pt7[:, i, :]
                            pt_cur.append(pt)
                        else:
                            pt = pts.tile([128, 1024], PDT, tag="pt")
                            pt_cur.append(pt)
                        if EXPS:
                            nc.scalar.activation(out=pt[:, 0:512], in_=s2e,
                                                 func=Exp)
                            nc.scalar.activation(out=pt[:, 512:1024], in_=s2o,
                                                 func=Exp)
                        else:
                            nc.scalar.activation(out=pt, in_=s2, func=Exp)
                        # DRAV: front-load the 8 AV pairs into chunks 0..7 and
                        # finish the sweep right after pair 7 — frees the psB
                        # ring half a slot early so the next sweep's first AV
                        # never waits on the normalize chain.
                        if prev is not None and (i < NK // 2 if DRAV else True):
                            if i == 0:
                                oe_ps = psB.tile([128, 512], F32, tag="psB",
                                                 name="oe_ps")
                                oo_ps = psB.tile([128, 512], F32, tag="psB",
                                                 name="oo_ps")
                            pw_par = prev[2]
                            emit_av_group(prev[:2], pw_par, pt_prev[i], i,
                                          oe_ps, oo_ps)
                            if DRAV and i == NK // 2 - 1:
                                finish_sweep(prev[0], prev[1], prev[2],
                                             oe_ps, oo_ps)
                                fin_done = True
                        for e in extras[i * ne // 16:(i + 1) * ne // 16]:
                            e()
                    if prev is not None and not (DRAV and fin_done):
                        finish_sweep(prev[0], prev[1], prev[2], oe_ps, oo_ps)
                    pt_prev = pt_cur
                    prev = (cur[0], cur[1], p)
                return pt_prev, prev

            def emit_drain(pt_prev, prev):
                """AV + normalize + y-projection of the final sweep."""
                oe_ps = psB.tile([128, 512], F32, tag="psB", name="oe_ps")
                oo_ps = psB.tile([128, 512], F32, tag="psB", name="oo_ps")
                for i in range(NK // 2 if DRAV else NK):
                    emit_av_group(prev[:2], prev[2], pt_prev[i], i, oe_ps, oo_ps)
                finish_sweep(prev[0], prev[1], prev[2], oe_ps, oo_ps)
                for pyn_ in ((2, 3) if PYL else (3,)):
                    for u in range(8):
                        py_unit_thunk(pyn_, u, prev[2])()

            # ---- prologue: rep 0 with a plain projection phase ----
            for n_ in range(NQ):
                xts0 = proj_loads(n_)
                for r_ in range(8):
                    for th in proj_round_thunks(n_, r_, xts0, 0):
                        th()
            pt_w, prev_w = emit_slots(0, None, None)

            # ---- steady-state bodies ----
            # unroll 4 bodies per For_i iteration: the loop back edge carries
            # an all-engine barrier (~15us), so amortize it over more reps
            nb = reps - 1
            U = int(os.environ.get("KUNROLL", "8"))

            def body(p):
                nonlocal pt_w, prev_w
                pt_w, prev_w = emit_slots(p, pt_w, prev_w)

            if nb >= U:
                with tc.For_i(0, nb // U, 1,
                              hint_engines=(mybir.EngineType.PE,
                                            mybir.EngineType.SP)):
                    for u_ in range(U):
                        body(1 - u_ % 2)
            for u_ in range(nb % U):
                body(1 - u_ % 2)

            # ---- epilogue: drain last sweep ----
            emit_drain(pt_w, prev_w)

            if part == "proj":
                assert not DRAV, "KPART=proj debug dump not supported with DRAV"
                for z in range(4):
                    nc.gpsimd.dma_start(
                        out=out_d[128 * z:128 * (z + 1), :],
                        in_=q_p[0][:, z // 2, 1024 * (z % 2):1024 * (z % 2 + 1)])
                    nc.gpsimd.dma_start(
                        out=out_d[128 * (4 + z):128 * (5 + z), :],
                        in_=k_p[0][:, z // 2, 1024 * (z % 2):1024 * (z % 2 + 1)])
                    vf = yts.tile([128, 4, 256], F32, tag="vf")
                    nc.vector.tensor_copy(out=vf, in_=v_p[0][:, 4 * z:4 * z + 4, :])
                    nc.sync.dma_start(
                        out=out_d[128 * (8 + z):128 * (9 + z), :].rearrange(
                            "p (a b) -> p a b", a=4),
                        in_=vf)
            elif part == "attn":
                for z in range(4):
                    onf = yts.tile([128, 1024], F32, tag="onf")
                    nc.vector.tensor_copy(
                        out=onf,
                        in_=on_p[0][:, z // 2, 1024 * (z % 2):1024 * (z % 2 + 1)])
                    nc.sync.dma_start(
                        out=out_d[128 * z:128 * (z + 1), :], in_=onf)

    nc.finalize()
    return nc


_NC = None


def _get_nc():
    global _NC
    if _NC is None:
        _NC = build_nc()
    return _NC


def make_in_maps(x, qkv_w, qkv_b, proj_w):
    """Host-side shard prep. Core c = 4*b + g handles batch b, heads 4g..4g+3."""
    x = np.asarray(x, np.float32)
    qkv_w = np.asarray(qkv_w, np.float32)
    qkv_b = np.asarray(qkv_b, np.float32)
    proj_w = np.asarray(proj_w, np.float32)
    in_maps = []
    onec = np.ones((128, 1), dtype=ml_dtypes.float8_e4m3 if P8
                   else ml_dtypes.bfloat16)
    for c in range(8):
        b, g = divmod(c, 4)
        hs = g * 4 * HD  # 256-wide feature slice for this core's heads
        xt = np.ascontiguousarray(x[b].T)                       # [C, N]
        wq = qkv_w[hs:hs + 256, :] * SCALE                      # pre-scaled q
        wk = qkv_w[C + hs:C + hs + 256, :]
        wqkT = np.ascontiguousarray(np.concatenate([wq, wk], 0).T)   # [C, 512]
        wqk = np.ascontiguousarray(wqkT.reshape(NJ, 128, 512).transpose(1, 0, 2))
        wvT = np.ascontiguousarray(qkv_w[2 * C + hs:2 * C + hs + 256, :].T)
        wv = np.ascontiguousarray(wvT.reshape(NJ, 128, 256).transpose(1, 0, 2))
        qb = np.ascontiguousarray((qkv_b[hs:hs + 256] * SCALE).reshape(2, 128).T)
        pwT = np.ascontiguousarray(proj_w[:, hs:hs + 256].T)    # [256, C]
        pw = np.ascontiguousarray(pwT.reshape(2, 128, 1024).transpose(1, 0, 2))
        if MM_BF16:
            bf = ml_dtypes.bfloat16
            xt, wqk, wv, pw = (a.astype(bf) for a in (xt, wqk, wv, pw))
        pdt_np = ml_dtypes.float8_e4m3 if P8 else ml_dtypes.bfloat16
        if DRAV:
            vones = np.zeros((128, NK * 4, VD - 64), dtype=pdt_np)
            vones[:, :, ONES_COL - 64] = 1
        else:
            vones = np.ones((128, NK, 4, 1), dtype=pdt_np)
        in_maps.append({"xt": xt, "wqk": wqk, "wv": wv, "qb": qb, "pw": pw,
                        "onec": onec, "vones": vones})
    return in_maps


def unshard(results, qkv_b, proj_w, proj_b):
    cvec = (np.asarray(qkv_b, np.float32)[2 * C:] @ np.asarray(proj_w, np.float32).T
            + np.asarray(proj_b, np.float32))
    y = np.empty((B, N, C), np.float32)
    for b in range(B):
        acc = results[4 * b]["out"].copy()
        for g in range(1, 4):
            acc += results[4 * b + g]["out"]
        y[b] = acc + cvec[None, :]
    return y


def kernel(x, qkv_w, qkv_b, proj_w, proj_b):
    nc = _get_nc()
    in_maps = make_in_maps(x, qkv_w, qkv_b, proj_w)
    res = run_bass_kernel_spmd(nc, in_maps, core_ids=list(range(8)))
    return unshard(res.results, qkv_b, proj_w, proj_b)



# revision 49
# speedup vs baseline: 1.0347x; 1.0347x over previous
"""Trainium2 Bass kernel for nn_MultiHeadSelfAttention (B=2, N=2048, C=1024, H=16).

Sharding: 8 cores = (batch b in {0,1}) x (head-group g in {0..3}); each core
computes 4 heads of one batch plus its partial output projection. The host
sums the 4 partial projections per batch and adds the bias constant
(v-bias and proj-bias folded together; k-bias is softmax-invariant and
dropped; q-bias applied on device).

Layouts (all transposed so no on-chip transposes needed):
  qT,kT [256,2048] = W_{q,k} @ x^T   (bf16)
  v     [2048,256] = x @ W_v^T       (fp8e4 + fp8 residual rows)
  S'_h  [kv,q] = K_h q_h^T           (K=64 row-tiled pairs, concurrent on PE)
  P'    = exp(S')                    (ACT, psum->sbuf, fp8e4)
  O^T_h [VD,q] = V_h^T P'_h          (fp8 DoubleRow: 2 kv-chunks per matmul,
                                      rows 0:64 = O, 64:96 = V-residual
                                      correction, 96 = softmax denominator)

Schedule: the attention sweeps s=(n,hp) run as 8 pipelined slots per rep;
slot t interleaves, per kv chunk i, the QK+exp of sweep t with the AV +
denominator matmuls of sweep t-1 (slot 0 consumes the previous rep's last
sweep), so the Scalar engine (exp) never drains. The QKV projection of rep
r+1 and the y-projection units ride the slots as fine-grained extra matmuls,
with q/k/v and O buffers double-buffered by rep parity. The rep loop is a
hardware For_i over an unrolled parity pair; a prologue rep (internal lag)
fills the pipeline and an epilogue drains the last sweep.
"""
import sys
import os

sys.path.insert(0, "/opt/trn_rl_repo")

import numpy as np
import ml_dtypes

import concourse.bass as bass
import concourse.mybir as mybir
from concourse import bacc
from concourse.tile import TileContext
from concourse import bass_utils
from concourse.bass_utils import run_bass_kernel_spmd

if os.environ.get("KLDWOPT", "0") == "1":
    # walrus ldw-opt: let the backend dedupe/hoist LDWEIGHTS
    _orig_run_command = bass_utils.run_command

    def _patched_run_command(argv, **kwargs):
        argv = ["--enable-ldw-opt=true" if a == "--enable-ldw-opt=false" else a
                for a in argv]
        return _orig_run_command(argv, **kwargs)

    bass_utils.run_command = _patched_run_command

F32R = mybir.dt.float32r
F32 = mybir.dt.float32
BF16 = mybir.dt.bfloat16
MM_BF16 = os.environ.get("KMMDT", "bf16") == "bf16"
MMDT = BF16 if MM_BF16 else F32R
# DoubleRow AV: P (exp scores) and V in fp8e4, kv-chunks paired so each AV
# matmul contracts 256 kv (2 fp8 weights/cell, 2 mul/cycle) — halves AV's
# PE stream time. Requires fp8 P+V (adds ~1.5e-2 rel err vs bf16's 2.5e-3;
# gate is 2e-2).
DRAV = os.environ.get("KDRAV", "1") == "1"
P8 = DRAV or os.environ.get("KP8", "0") == "1"
PDT = mybir.dt.float8e4 if P8 else BF16
DR = mybir.MatmulPerfMode.DoubleRow
# V-residual: AV lhsT rows 64:96 carry res = v - fp8(v) for features 0:31,
# added back to O at normalize time — free in the matmul (cost ~ N columns,
# independent of M); halves the fp8-V error variance. Row 96 = ones (D).
# 32-row blocks keep every DVE read quadrant-aligned (walrus requirement).
# VRES on: with AV front-loading, the VD=100 + residual config measured
# fastest (182.0/184.6us vs 189.1 at VD=80) — the lighter finish_sweep at
# VD=80 interacts badly with the early finish; and VRES buys rel err
# 1.56e-2 vs 1.72e-2 (gate 2e-2).
VRES = DRAV and os.environ.get("KVRES", "1") == "1"
# VD: lhsT free size per pair half (64 v-feats + ones + pad to a 16B-aligned
# pair stride: 4*VD % 16 == 0). Smaller VD = fewer LDWEIGHTS columns.
VD = 100 if VRES else int(os.environ.get("KVD", "80"))
ONES_COL = 96 if VRES else 64
# GPSIMD cannot access PSUM (walrus BIR rule) — evictions read PSUM, so
# they must stay on DVE.
GPEV = os.environ.get("KGPEV", "0") == "1"
# Split per-head [128,512] exps + psA=5x1-bank + psC=1: measured 297us on HW
# (vs 184.6) — the 1-deep psC ring stalls the interleaved projection chains.
# Keep off.
EXPS = DRAV and os.environ.get("KEXPS", "0") == "1"
# y-proj groups 2 slots later than earliest-ready: measured 188.1us vs 182.5
# — the JIT placement is fine (the wrap units on slots 1,3 collide with the
# proj interleave). Keep off.
PYL = os.environ.get("KPYL", "0") == "1"
# SBUF->SBUF partition-broadcast DMA is illegal (zero partition step) — the
# DRAM round trip through dscr is required for the denominator broadcast.
SBBC = os.environ.get("KSBBC", "0") == "1"
Exp = mybir.ActivationFunctionType.Exp

B, N, C, H = 2, 2048, 1024, 16
HD = C // H          # 64
SCALE = 1.0 / np.sqrt(HD).astype(np.float32)

NQ = N // 512        # 4 q-chunks of 512
NK = N // 128        # 16 kv-chunks of 128
NJ = C // 128        # 8 contraction chunks for projections


def build_nc():
    part = os.environ.get("KPART", "full")
    reps = int(os.environ.get("KREPS", "1"))
    nc = bacc.Bacc("TRN2", target_bir_lowering=False, debug=False, num_devices=8)

    xt_d = nc.dram_tensor("xt", [C, N], MMDT, kind="ExternalInput").ap()
    wqk_d = nc.dram_tensor("wqk", [128, NJ, 512], MMDT, kind="ExternalInput").ap()
    wv_d = nc.dram_tensor("wv", [128, NJ, 256], MMDT, kind="ExternalInput").ap()
    qb_d = nc.dram_tensor("qb", [128, 2], F32, kind="ExternalInput").ap()
    pw_d = nc.dram_tensor("pw", [128, 2, 1024], MMDT, kind="ExternalInput").ap()
    one_d = nc.dram_tensor("onec", [128, 1], PDT, kind="ExternalInput").ap()
    if DRAV:
        # per (chunk-pair, pair, head): feature tail 64:VD = [1, 0*(VD-65)]
        vones_d = nc.dram_tensor("vones", [128, NK * 4, VD - 64], PDT,
                                 kind="ExternalInput").ap()
    else:
        vones_d = nc.dram_tensor("vones", [128, NK, 4, 1], PDT,
                                 kind="ExternalInput").ap()
    out_d = nc.dram_tensor("out", [N, C], F32, kind="ExternalOutput").ap()

    ilv = reps > 1  # interleave next-rep projection into the slots

    with TileContext(nc) as tc:
        with tc.tile_pool(name="const", bufs=1) as const, \
             tc.tile_pool(name="persist", bufs=1) as persist, \
             tc.tile_pool(name="xs", bufs=16) as xs, \
             tc.tile_pool(name="pts", bufs=28) as pts, \
             tc.tile_pool(name="dsbp", bufs=1) as dsbp, \
             tc.tile_pool(name="dbp", bufs=1) as dbp, \
             tc.tile_pool(name="rbp", bufs=1) as rbp, \
             tc.tile_pool(name="oup", bufs=2) as oup, \
             tc.tile_pool(name="yts", bufs=3) as yts, \
             tc.tile_pool(name="scr", bufs=1, space="DRAM") as scr, \
             tc.tile_pool(name="psA", bufs=(5 if EXPS else 2),
                          space="PSUM") as psA, \
             tc.tile_pool(name="psB", bufs=2, space="PSUM") as psB, \
             tc.tile_pool(name="psC", bufs=(1 if EXPS else 2),
                          space="PSUM") as psC:

            wqk_t = const.tile([128, NJ, 512], MMDT)
            wv_t = const.tile([128, NJ, 256], MMDT)
            qb_t = const.tile([128, 2], F32)
            pw_t = const.tile([128, 2, 1024], MMDT)
            ones_t = const.tile([128, 1], PDT)

            npar = 2 if ilv else 1
            q_p = [persist.tile([128, 2, N], MMDT, name=f"q_all{i_}")
                   for i_ in range(npar)]
            k_p = [persist.tile([128, 2, N], MMDT, name=f"k_all{i_}")
                   for i_ in range(npar)]
            # [kv, chunk, head(4), 64 v-feats + ones]: the ones column makes
            # the AV matmul (M=65) emit the softmax denominator as row 64.
            # DoubleRow: [kv, chunk-pair, pair, head, VD] — 64 feats + ones +
            # (residual rows | pad); pair stride (4*VD B) stays 16B-aligned.
            if DRAV:
                v_p = [persist.tile([128, NK // 2, 2, 4, VD], PDT,
                                    name=f"v_all{i_}") for i_ in range(npar)]
            else:
                v_p = [persist.tile([128, NK, 4, 65], PDT, name=f"v_all{i_}")
                       for i_ in range(npar)]
            on_p = [persist.tile([128, 2, N], MMDT, name=f"on_all{i_}")
                    for i_ in range(npar)]
            # slot-7 exp output crosses the For_i back edge (consumed by the
            # next rep's slot 0), so it lives in a persistent ring, not a pool
            if DRAV:
                pt7 = persist.tile([128, NK // 2, 2, 1024], PDT, name="pt7")
            else:
                pt7 = persist.tile([128, NK, 1024], PDT, name="pt7")
            dscr = [scr.tile([2, NQ, 2, 512], F32, name=f"dscr{i_}")
                    for i_ in range(npar)]

            # constants load once
            nc.scalar.dma_start(out=qb_t, in_=qb_d)
            for par_ in range(npar):
                if DRAV:
                    nc.scalar.dma_start(
                        out=v_p[par_][:, :, :, :, 64:VD].rearrange(
                            "p a b c d -> p (a b c) d"),
                        in_=vones_d)
                else:
                    nc.scalar.dma_start(out=v_p[par_][:, :, :, 64:65],
                                        in_=vones_d)
            nc.scalar.dma_start(out=ones_t, in_=one_d)
            nc.scalar.dma_start(out=pw_t, in_=pw_d)
            for j in range(NJ):
                nc.scalar.dma_start(out=wqk_t[:, j, :], in_=wqk_d[:, j, :])
                nc.scalar.dma_start(out=wv_t[:, j, :], in_=wv_d[:, j, :])

            def proj_loads(n):
                xts = []
                for j in range(NJ):
                    xt_t = xs.tile([128, 512], MMDT, tag="xt")
                    nc.sync.dma_start(
                        out=xt_t,
                        in_=xt_d[128 * j:128 * (j + 1), 512 * n:512 * (n + 1)])
                    xts.append(xt_t)
                return xts

            def proj_round_thunks(n, r, xts, par):
                """Chain r (of 8) of chunk n: ONE accumulation chain in ONE
                PSUM bank, as single-MM thunks + an eviction thunk. Single-
                bank chains alternate psC bufs, so each chain's eviction
                hides behind the next chain instead of head-blocking it."""
                nsl = slice(512 * n, 512 * (n + 1))
                c = psC.tile([128, 512], F32, tag="psC", name=f"pr{r}")
                thunks = []
                for j in range(NJ):
                    st, sp = (j == 0), (j == NJ - 1)
                    if r < 4:
                        base = 128 * r

                        def mm(j=j, base=base, st=st, sp=sp):
                            nc.tensor.matmul(
                                c, lhsT=wqk_t[:, j, base:base + 128],
                                rhs=xts[j], start=st, stop=sp)
                    else:
                        t_ = r - 4

                        def mm(j=j, t_=t_, st=st, sp=sp):
                            nc.tensor.matmul(
                                c[:, 0:256],
                                lhsT=xts[j][:, 128 * t_:128 * (t_ + 1)],
                                rhs=wv_t[:, j, :], start=st, stop=sp)
                    thunks.append(mm)

                # proj evictions run on GpSimd (otherwise idle): keeps DVE
                # free for the normalize chain and unblocks the psC ring
                # sooner when DVE's queue is backed up.
                ev = nc.gpsimd if GPEV else nc.vector

                def evict():
                    if r < 2:
                        ev.tensor_scalar_add(
                            out=q_p[par][:, r, nsl], in0=c,
                            scalar1=qb_t[:, r:r + 1])
                    elif r < 4:
                        ev.tensor_copy(out=k_p[par][:, r - 2, nsl],
                                       in_=c)
                    else:
                        j_ = 4 * n + (r - 4)
                        dst = (v_p[par][:, j_ // 2, j_ % 2, :, 0:64]
                               if DRAV else v_p[par][:, j_, :, 0:64])
                        src = c[:, 0:256].rearrange("p (a b) -> p a b", a=4)
                        ev.tensor_copy(out=dst, in_=src)
                        if VRES:
                            vsl = v_p[par][:, j_ // 2, j_ % 2]
                            ev.tensor_sub(
                                out=vsl[:, :, 64:96],
                                in0=src[:, :, 0:32],
                                in1=vsl[:, :, 0:32])
                thunks.append(evict)
                return thunks

            def py_unit_thunk(pyn, u, par):
                def run():
                    m = 4 * pyn + u // 2
                    nn = u % 2
                    py = psC.tile([128, 512], F32, tag="psC", name="py")
                    for hp in range(2):
                        nc.tensor.matmul(
                            py, lhsT=on_p[par][:, hp, 128 * m:128 * (m + 1)],
                            rhs=pw_t[:, hp, 512 * nn:512 * (nn + 1)],
                            start=(hp == 0), stop=(hp == 1))
                    yt = yts.tile([128, 512], F32, tag="yt")
                    nc.vector.tensor_copy(out=yt, in_=py)
                    nc.sync.dma_start(
                        out=out_d[128 * m:128 * (m + 1),
                                  512 * nn:512 * (nn + 1)],
                        in_=yt)
                return run

            def finish_sweep(pn, php, par, oe_ps, oo_ps):
                pnsl = slice(512 * pn, 512 * (pn + 1))
                # bank A's readers (ou rows + D row) run first on DVE so the
                # next slot's first AV matmul (head-even -> bank A) unblocks
                # ~0.7us earlier; bank B's reader follows
                ou = oup.tile([128, 512], F32, tag="ou")
                dsb = dsbp.tile([128, 512], F32, tag="dsb")
                osh = dsbp.tile([128, 512], F32, tag="osh")
                if VRES:
                    # O = hi + res (features 0:31); res rows live at PSUM
                    # partitions 64:96, staged through an aligned copy (a
                    # partition-shifted two-input add is illegal).
                    rse = dsbp.tile([128, 512], F32, tag="rse")
                    nc.vector.tensor_copy(out=rse[0:32, :], in_=oe_ps[64:96, :])
                    nc.vector.tensor_add(out=ou[0:32, :], in0=oe_ps[0:32, :],
                                         in1=rse[0:32, :])
                    nc.vector.tensor_copy(out=ou[32:64, :], in_=oe_ps[32:64, :])
                    nc.vector.tensor_copy(out=dsb[64:65, :], in_=oe_ps[96:97, :])
                    rso = dsbp.tile([128, 512], F32, tag="rso")
                    nc.vector.tensor_copy(out=rso[0:32, :], in_=oo_ps[64:96, :])
                    nc.vector.tensor_add(out=osh[0:32, :], in0=oo_ps[0:32, :],
                                         in1=rso[0:32, :])
                    nc.vector.tensor_copy(out=osh[32:64, :], in_=oo_ps[32:64, :])
                    nc.vector.tensor_copy(out=osh[64:65, :], in_=oo_ps[96:97, :])
                else:
                    nc.vector.tensor_copy(out=ou[0:64, :], in_=oe_ps[0:64, :])
                    nc.vector.tensor_copy(out=dsb[64:65, :], in_=oe_ps[64:65, :])
                    nc.vector.tensor_copy(out=osh[0:65, :], in_=oo_ps[0:65, :])
                nc.sync.dma_start(out=ou[64:128, :], in_=osh[0:64, :])
                db = dbp.tile([128, 512], F32, tag="db")
                if SBBC:
                    # broadcast the denominator rows SBUF->SBUF directly
                    nc.sync.dma_start(
                        out=db[0:64, :],
                        in_=dsb[64:65, :].to_broadcast([64, 512]))
                    nc.sync.dma_start(
                        out=db[64:128, :],
                        in_=osh[64:65, :].to_broadcast([64, 512]))
                else:
                    nc.sync.dma_start(out=dscr[par][php, pn, 0, :],
                                      in_=dsb[64:65, :])
                    nc.sync.dma_start(out=dscr[par][php, pn, 1, :],
                                      in_=osh[64:65, :])
                    nc.sync.dma_start(
                        out=db[0:64, :],
                        in_=dscr[par][php, pn, 0:1, :].to_broadcast([64, 512]))
                    nc.sync.dma_start(
                        out=db[64:128, :],
                        in_=dscr[par][php, pn, 1:2, :].to_broadcast([64, 512]))
                rb = rbp.tile([128, 512], F32, tag="rb")
                nc.vector.reciprocal_approx_fast(out=rb, in_=db)
                # keep the multiply on DVE: GpSimd's per-op dispatch overhead
                # made it a net loss (measured +~10us)
                nc.vector.tensor_mul(out=on_p[par][:, php, pnsl], in0=ou,
                                     in1=rb)

            def emit_av_group(prev, ppar, ptp, i, oe_ps, oo_ps):
                pn, php = prev
                if DRAV:
                    # ptp: [128, 2, 1024] fp8 pair tile; i is the pair index.
                    st, sp = (i == 0), (i == NK // 2 - 1)
                    nc.tensor.matmul(oe_ps[0:VD, :],
                                     lhsT=v_p[ppar][:, i, :, 2 * php, 0:VD],
                                     rhs=ptp[:, :, 0:512],
                                     start=st, stop=sp, perf_mode=DR)
                    nc.tensor.matmul(oo_ps[0:VD, :],
                                     lhsT=v_p[ppar][:, i, :, 2 * php + 1, 0:VD],
                                     rhs=ptp[:, :, 512:1024],
                                     start=st, stop=sp, perf_mode=DR)
                    return
                st, sp = (i == 0), (i == NK - 1)
                nc.tensor.matmul(oe_ps[0:65, :],
                                 lhsT=v_p[ppar][:, i, 2 * php, 0:65],
                                 rhs=ptp[:, 0:512], start=st, stop=sp)
                nc.tensor.matmul(oo_ps[0:65, :],
                                 lhsT=v_p[ppar][:, i, 2 * php + 1, 0:65],
                                 rhs=ptp[:, 512:1024], start=st, stop=sp)

            def emit_slots(p, pt_wrap, wrap_prev):
                """8 pipelined slots of one rep (parity p).

                pt_wrap/wrap_prev: pt list + (sweep, parity) whose AV runs in
                slot 0 (the previous rep's sweep 7), or None for internal lag
                (prologue: slot 0 has no AV).
                Returns the pt list + sweep of slot 7 (for the next rep/drain).
                """
                pt_prev, prev = pt_wrap, wrap_prev
                for t in range(8):
                    cur = (t // 2, t % 2)
                    n, hp = cur
                    nsl = slice(512 * n, 512 * (n + 1))
                    # extras for this slot
                    extras = []
                    if ilv:
                        ch = t // 2   # proj chunk rides slots 2ch, 2ch+1
                        if t % 2 == 0:
                            if getattr(emit_slots, "pre_ch", None) == ch:
                                xts_c = emit_slots.pre
                            else:
                                xts_c = proj_loads(ch)
                            emit_slots.xts = xts_c
                            for r in range(6):
                                extras += proj_round_thunks(ch, r, xts_c, 1 - p)
                        else:
                            for r in (6, 7):
                                extras += proj_round_thunks(ch, r,
                                                            emit_slots.xts,
                                                            1 - p)
                            if ch < 3:
                                # prefetch next chunk's x tiles a slot early
                                # so its first proj matmul never waits on DMA
                                emit_slots.pre = proj_loads(ch + 1)
                                emit_slots.pre_ch = ch + 1
                    if PYL:
                        # y-proj groups shifted 2 slots later than their
                        # earliest-ready point: pyn 0,1 at slots 5,7 (>=3
                        # slots after their on_p normalize completes), pyn
                        # 2,3 ride the NEXT rep's slots 1,3 (a full rep of
                        # slack) — the old placement was exactly just-in-
                        # time behind the dscr DRAM round trip.
                        if t in (1, 3) and wrap_prev is not None:
                            for u in range(8):
                                extras.append(py_unit_thunk(
                                    2 + (t - 1) // 2, u, wrap_prev[2]))
                        if t in (5, 7):
                            for u in range(8):
                                extras.append(py_unit_thunk((t - 5) // 2,
                                                            u, p))
                    else:
                        if t == 1 and wrap_prev is not None:
                            for u in range(8):
                                extras.append(py_unit_thunk(3, u,
                                                            wrap_prev[2]))
                        if t in (3, 5, 7):
                            for u in range(8):
                                extras.append(py_unit_thunk((t - 3) // 2,
                                                            u, p))
                    ne = len(extras)

                    pt_cur = []
                    oe_ps = oo_ps = None
                    ptp = None
                    fin_done = False
                    for i in range(16):
                        isl = slice(128 * i, 128 * (i + 1))
                        if EXPS:
                            s2e = psA.tile([128, 512], F32, tag="psA",
                                           name="s2e")
                            s2o = psA.tile([128, 512], F32, tag="psA",
                                           name="s2o")
                            nc.tensor.matmul(s2e,
                                             lhsT=k_p[p][0:64, hp, isl],
                                             rhs=q_p[p][0:64, hp, nsl],
                                             start=True, stop=True)
                            nc.tensor.matmul(s2o,
                                             lhsT=k_p[p][64:128, hp, isl],
                                             rhs=q_p[p][64:128, hp, nsl],
                                             start=True, stop=True)
                        else:
                            s2 = psA.tile([128, 1024], F32, tag="psA",
                                          name="s2")
                            nc.tensor.matmul(s2[:, 0:512],
                                             lhsT=k_p[p][0:64, hp, isl],
                                             rhs=q_p[p][0:64, hp, nsl],
                                             start=True, stop=True)
                            nc.tensor.matmul(s2[:, 512:1024],
                                             lhsT=k_p[p][64:128, hp, isl],
                                             rhs=q_p[p][64:128, hp, nsl],
                                             start=True, stop=True)
                        if DRAV:
                            if i % 2 == 0:
                                if t == 7:
                                    ptp = pt7[:, i // 2]
                                else:
                                    ptp = pts.tile([128, 2, 1024], PDT,
                                                   tag="pt")
                                pt_cur.append(ptp)
                            pt = ptp[:, i % 2, :]
                        elif t == 7:
                            pt = pt7[:, i, :]
                            pt_cur.append(pt)
                        else:
                            pt = pts.tile([128, 1024], PDT, tag="pt")
                            pt_cur.append(pt)
                        if EXPS:
                            nc.scalar.activation(out=pt[:, 0:512], in_=s2e,
                                                 func=Exp)
                            nc.scalar.activation(out=pt[:, 512:1024], in_=s2o,
                                                 func=Exp)
                        else:
                            nc.scalar.activation(out=pt, in_=s2, func=Exp)
                        # DRAV: front-load the 8 AV pairs into chunks 0..7 and
                        # finish the sweep right after pair 7 — frees the psB
                        # ring half a slot early so the next sweep's first AV
                        # never waits on the normalize chain.
                        if prev is not None and (i < NK // 2 if DRAV else True):
                            if i == 0:
                                oe_ps = psB.tile([128, 512], F32, tag="psB",
                                                 name="oe_ps")
                                oo_ps = psB.tile([128, 512], F32, tag="psB",
                                                 name="oo_ps")
                            pw_par = prev[2]
                            emit_av_group(prev[:2], pw_par, pt_prev[i], i,
                                          oe_ps, oo_ps)
                            if DRAV and i == NK // 2 - 1:
                                finish_sweep(prev[0], prev[1], prev[2],
                                             oe_ps, oo_ps)
                                fin_done = True
                        for e in extras[i * ne // 16:(i + 1) * ne // 16]:
                            e()
                    if prev is not None and not (DRAV and fin_done):
                        finish_sweep(prev[0], prev[1], prev[2], oe_ps, oo_ps)
                    pt_prev = pt_cur
                    prev = (cur[0], cur[1], p)
                return pt_prev, prev

            def emit_drain(pt_prev, prev):
                """AV + normalize + y-projection of the final sweep."""
                oe_ps = psB.tile([128, 512], F32, tag="psB", name="oe_ps")
                oo_ps = psB.tile([128, 512], F32, tag="psB", name="oo_ps")
                for i in range(NK // 2 if DRAV else NK):
                    emit_av_group(prev[:2], prev[2], pt_prev[i], i, oe_ps, oo_ps)
                finish_sweep(prev[0], prev[1], prev[2], oe_ps, oo_ps)
                for pyn_ in ((2, 3) if PYL else (3,)):
                    for u in range(8):
                        py_unit_thunk(pyn_, u, prev[2])()

            # ---- prologue: rep 0 with a plain projection phase ----
            for n_ in range(NQ):
                xts0 = proj_loads(n_)
                for r_ in range(8):
                    for th in proj_round_thunks(n_, r_, xts0, 0):
                        th()
            pt_w, prev_w = emit_slots(0, None, None)

            # ---- steady-state bodies ----
            # unroll 4 bodies per For_i iteration: the loop back edge carries
            # an all-engine barrier (~15us), so amortize it over more reps
            nb = reps - 1
            U = int(os.environ.get("KUNROLL", "8"))

            def body(p):
                nonlocal pt_w, prev_w
                pt_w, prev_w = emit_slots(p, pt_w, prev_w)

            if nb >= U:
                with tc.For_i(0, nb // U, 1,
                              hint_engines=(mybir.EngineType.PE,
                                            mybir.EngineType.SP)):
                    for u_ in range(U):
                        body(1 - u_ % 2)
            for u_ in range(nb % U):
                body(1 - u_ % 2)

            # ---- epilogue: drain last sweep ----
            emit_drain(pt_w, prev_w)

            if part == "proj":
                assert not DRAV, "KPART=proj debug dump not supported with DRAV"
                for z in range(4):
                    nc.gpsimd.dma_start(
                        out=out_d[128 * z:128 * (z + 1), :],
                        in_=q_p[0][:, z // 2, 1024 * (z % 2):1024 * (z % 2 + 1)])
                    nc.gpsimd.dma_start(
                        out=out_d[128 * (4 + z):128 * (5 + z), :],
                        in_=k_p[0][:, z // 2, 1024 * (z % 2):1024 * (z % 2 + 1)])
                    vf = yts.tile([128, 4, 256], F32, tag="vf")
                    nc.vector.tensor_copy(out=vf, in_=v_p[0][:, 4 * z:4 * z + 4, :])
                    nc.sync.dma_start(
                        out=out_d[128 * (8 + z):128 * (9 + z), :].rearrange(
                            "p (a b) -> p a b", a=4),
                        in_=vf)
            elif part == "attn":
                for z in range(4):
                    onf = yts.tile([128, 1024], F32, tag="onf")
                    nc.vector.tensor_copy(
                        out=onf,
                        in_=on_p[0][:, z // 2, 1024 * (z % 2):1024 * (z % 2 + 1)])
                    nc.sync.dma_start(
                        out=out_d[128 * z:128 * (z + 1), :], in_=onf)

    nc.finalize()
    return nc


_NC = None


def _get_nc():
    global _NC
    if _NC is None:
        _NC = build_nc()
    return _NC


def make_in_maps(x, qkv_w, qkv_b, proj_w):
    """Host-side shard prep. Core c = 4*b + g handles batch b, heads 4g..4g+3."""
    x = np.asarray(x, np.float32)
    qkv_w = np.asarray(qkv_w, np.float32)
    qkv_b = np.asarray(qkv_b, np.float32)
    proj_w = np.asarray(proj_w, np.float32)
    in_maps = []
    onec = np.ones((128, 1), dtype=ml_dtypes.float8_e4m3 if P8
                   else ml_dtypes.bfloat16)
    for c in range(8):
        b, g = divmod(c, 4)
        hs = g * 4 * HD  # 256-wide feature slice for this core's heads
        xt = np.ascontiguousarray(x[b].T)                       # [C, N]
        wq = qkv_w[hs:hs + 256, :] * SCALE                      # pre-scaled q
        wk = qkv_w[C + hs:C + hs + 256, :]
        wqkT = np.ascontiguousarray(np.concatenate([wq, wk], 0).T)   # [C, 512]
        wqk = np.ascontiguousarray(wqkT.reshape(NJ, 128, 512).transpose(1, 0, 2))
        wvT = np.ascontiguousarray(qkv_w[2 * C + hs:2 * C + hs + 256, :].T)
        wv = np.ascontiguousarray(wvT.reshape(NJ, 128, 256).transpose(1, 0, 2))
        qb = np.ascontiguousarray((qkv_b[hs:hs + 256] * SCALE).reshape(2, 128).T)
        pwT = np.ascontiguousarray(proj_w[:, hs:hs + 256].T)    # [256, C]
        pw = np.ascontiguousarray(pwT.reshape(2, 128, 1024).transpose(1, 0, 2))
        if MM_BF16:
            bf = ml_dtypes.bfloat16
            xt, wqk, wv, pw = (a.astype(bf) for a in (xt, wqk, wv, pw))
        pdt_np = ml_dtypes.float8_e4m3 if P8 else ml_dtypes.bfloat16
        if DRAV:
            vones = np.zeros((128, NK * 4, VD - 64), dtype=pdt_np)
            vones[:, :, ONES_COL - 64] = 1
        else:
            vones = np.ones((128, NK, 4, 1), dtype=pdt_np)
        in_maps.append({"xt": xt, "wqk": wqk, "wv": wv, "qb": qb, "pw": pw,
                        "onec": onec, "vones": vones})
    return in_maps


def unshard(results, qkv_b, proj_w, proj_b):
    cvec = (np.asarray(qkv_b, np.float32)[2 * C:] @ np.asarray(proj_w, np.float32).T
            + np.asarray(proj_b, np.float32))
    y = np.empty((B, N, C), np.float32)
    for b in range(B):
        acc = results[4 * b]["out"].copy()
        for g in range(1, 4):
            acc += results[4 * b + g]["out"]
        y[b] = acc + cvec[None, :]
    return y


def kernel(x, qkv_w, qkv_b, proj_w, proj_b):
    nc = _get_nc()
    in_maps = make_in_maps(x, qkv_w, qkv_b, proj_w)
    res = run_bass_kernel_spmd(nc, in_maps, core_ids=list(range(8)))
    return unshard(res.results, qkv_b, proj_w, proj_b)

